# revision 24
# baseline (speedup 1.0000x reference)
"""Trainium2 Bass kernel for nn_Attention_org_cross_85074712199395.

Reference computes two fully independent cross-attention branches:
  branch 0: Q(emb1;Wq)   x Kd,Vd(emb_alld;Wkd0,Wvd0) -> O1  via Wout
  branch 1: Qd(embd1;Wqd) x K,V  (emb_all; Wk0, Wv0 ) -> Od1 via Woutd
Sharding: 8 cores = 4 batches x 2 branches. Zero collectives.

v3: algebraic restructure + head-major streaming. Q/K/V projections fold
into small per-head sandwiches around the two N-sized contractions:

  G_h  = E1_h^T Ed_h                  [256,256] (1.07e9 MACs, DMA-bound)
  s_h  = Wq_h G_h Wkd^T               (tiny)
  p_h  = exp(inorm(s_h*scale))        (unnormalized; 1/den folded at pv)
  pv_h = (p_h^T)^T Wvd                [256,256] (tiny)
  M_h  = pv_h^T WoutT_h               [256,1024] (tiny)
  O    = Ed_cm^T M_stack              [4096,1024] (4.29e9 MACs)

The emb stream is ordered HEAD-major so G_h completes every ~14us and
head h's whole softmax/sandwich chain (evac/T2/s/stats/exp/p^T/pv) runs
on vector+scalar inside head h+1's DMA window, emitted via a generator
interleaved into the next head's G loop. Scalar runs only Sqrt+Exp
(tables preloaded; sum-of-squares via DVE mult+reduce) so there are no
activation-table swaps on the critical path. All matmuls fp16 (fp8
DoubleRow measured 2x but its ~4% element noise fails the 2e-2 gate).
"""

import sys
import types

import numpy as np

B, N, C, KV, H = 4, 4096, 1024, 1024, 4
Ch = C // H          # 256
EPS_ADJ = 1e-5 * KV  # InstanceNorm eps with the 1/sqrt(KV) score scale folded in
NP = N // 256        # 16 token slice-pairs per head for G
NG = N // 512        # 8 token groups for O
O_FP8 = False
G_FP8 = False


def _ensure_axon_hooks():
    """Inject antenv.axon_hooks (absent in this image) so trace=True works."""
    if "antenv.axon_hooks" in sys.modules:
        return
    try:
        import antenv  # noqa: F401
    except ImportError:
        return
    mod = types.ModuleType("antenv.axon_hooks")
    state = [None]
    mod.set_axon_ntff_profile_hook = lambda h: state.__setitem__(0, h)
    mod.get_axon_ntff_profile_hook = lambda: state[0]
    sys.modules["antenv.axon_hooks"] = mod
    try:
        from trn_agent_boot.trn_boot import _ntff_profile_via_ctypes

        mod.set_axon_ntff_profile_hook(
            _ntff_profile_via_ctypes("/opt/axon/libaxon_pjrt.so")
        )
    except Exception:
        pass


def build_nc(n_tokens=N):
    """Build + compile the per-core Bass program (SPMD-identical on all cores)."""
    import concourse.bass as bass
    import concourse.mybir as mybir
    import concourse.tile as tile
    from concourse import bacc
    from concourse.masks import make_identity

    f32 = mybir.dt.float32
    f16 = mybir.dt.float16
    Exp = mybir.ActivationFunctionType.Exp
    X = mybir.AxisListType.X
    mult = mybir.AluOpType.mult
    add = mybir.AluOpType.add
    np_ = n_tokens // 256
    ng = n_tokens // 512

    nc = bacc.Bacc("TRN2", target_bir_lowering=False, debug=False, num_devices=8)

    # head-major token-major embeddings: [p, h, slice-pair, (sl01, ch)]
    f8 = mybir.dt.float8e4
    DR = mybir.MatmulPerfMode.DoubleRow
    emb_dt = f8 if G_FP8 else f16
    cm_dt = f8 if O_FP8 else f16
    e1tm_d = nc.dram_tensor("e1tm", [128, H * np_ * 2, 256], emb_dt, kind="ExternalInput").ap()
    edtm_d = nc.dram_tensor("edtm", [128, H * np_ * 2, 256], emb_dt, kind="ExternalInput").ap()
    edcm_d = nc.dram_tensor("edcm", [128, 8, n_tokens], cm_dt, kind="ExternalInput").ap()
    wq_d = nc.dram_tensor("wq", [128, 8, 256], f16, kind="ExternalInput").ap()
    wk_d = nc.dram_tensor("wk", [128, 2, 256], f16, kind="ExternalInput").ap()
    wv_d = nc.dram_tensor("wv", [128, 2, 256], f16, kind="ExternalInput").ap()
    wout_d = nc.dram_tensor("wout", [128, 8, 1024], f16, kind="ExternalInput").ap()
    out_d = nc.dram_tensor("out", [n_tokens, 1024], f16, kind="ExternalOutput").ap()

    from contextlib import ExitStack

    with tile.TileContext(nc) as tc:
        with ExitStack() as ctx:
            pool = lambda **kw: ctx.enter_context(tc.tile_pool(**kw))
            wpool = pool(name="weights", bufs=1)
            pt_pool = pool(name="pt", bufs=1)
            den_pool = pool(name="den", bufs=1)
            gsb_pool = pool(name="gsb", bufs=1)
            t2sb_pool = pool(name="t2sb", bufs=1)
            pvsb_pool = pool(name="pvsb", bufs=1)
            p_pool = pool(name="psb", bufs=1)
            stat_pool = pool(name="stat", bufs=8)
            scr_pool = pool(name="scr", bufs=1)
            msb_pool = pool(name="msb", bufs=1)

            ident = wpool.tile([128, 128], f16)
            make_identity(nc, ident[:])
            ones_col = wpool.tile([128, 1], f32)
            nc.vector.memset(ones_col[:], 1.0)
            ones_row = wpool.tile([1, 128], f32)
            nc.vector.memset(ones_row[:], 1.0)
            warm_sb = wpool.tile([128, 128], f32)
            nc.vector.memset(warm_sb[:], 0.0)
            # preload the Sqrt and Exp activation tables off the critical path
            tdum = wpool.tile([1, 4], f32)
            nc.vector.memset(tdum[:1, 0:2], 1.0)
            nc.scalar.sqrt(tdum[:1, 2:3], tdum[:1, 0:1])
            nc.scalar.activation(tdum[:1, 3:4], tdum[:1, 1:2], Exp)

            wq = wpool.tile([128, 8, 256], f16)
            wk = wpool.tile([128, 2, 256], f16)
            wv = wpool.tile([128, 2, 256], f16)
            wout = wpool.tile([128, 8, 1024], f16)

            pts = [pt_pool.tile([128, 256], f16, name=f"pt{i}", tag=f"pt{i}")
                   for i in range(8)]                                  # p^T (k,c)
            dens = [den_pool.tile([128, 4], f32, name=f"den{i}", tag=f"den{i}")
                    for i in range(4)]
            g_sb = [gsb_pool.tile([128, 512], f16, name=f"g{i}", tag=f"g{i}")
                    for i in range(4)]                                 # G (c,k')
            t2_sb = [t2sb_pool.tile([128, 512], f16, name=f"t2{i}", tag=f"t2{i}")
                     for i in range(4)]                                # (Wq G) (k',oq)
            pv_sb = [pvsb_pool.tile([128, 512], f16, name=f"pv{i}", tag=f"pv{i}")
                     for i in range(4)]                                # p.Wvd (c,kk)
            ps = [p_pool.tile([128, 512], f16, name=f"p{i}", tag=f"p{i}")
                  for i in range(4)]
            m_sb = msb_pool.tile([128, 8, 1024], cm_dt)                # M (K,o)
            scratch = scr_pool.tile([128, 512], f32)

            with tc.tile_pool(name="warm_ps", bufs=1, space="PSUM") as warm_pool:
                wps = warm_pool.tile([128, 512], f32)
                for w in range(12):
                    nc.tensor.matmul(wps[:, 0:128], warm_sb[:], warm_sb[:],
                                     start=(w == 0), stop=(w == 11))
                # verifier requires a reader for every written PSUM location
                nc.vector.tensor_copy(tdum[:1, 0:1], wps[:1, 0:1])

            ekvB_pool = pool(name="ekvB", bufs=3)
            with ExitStack() as actx:
                apool = lambda **kw: actx.enter_context(tc.tile_pool(**kw))
                e1_pool = apool(name="e1A", bufs=3)
                s16_pool = apool(name="s16", bufs=2)
                ed_pool = apool(name="edA", bufs=3)
                g_ps = apool(name="g_ps", bufs=1, space="PSUM")
                t2_ps = apool(name="t2_ps", bufs=1, space="PSUM")
                s_ps = apool(name="s_ps", bufs=1, space="PSUM")
                tb_ps = apool(name="tb_ps", bufs=1, space="PSUM")
                pt_ps = apool(name="pt_ps", bufs=1, space="PSUM")
                pv_ps = apool(name="pv_ps", bufs=1, space="PSUM")
                m_ps = apool(name="m_ps", bufs=2, space="PSUM")

                inv = 1.0 / (256.0 * 256.0)

                def chain(h, g_tile):
                    """Per-head softmax/sandwich pipeline, emitted in 4 pieces."""
                    # --- P1: G evac, T2 = (Wq G) as [k',oq], s, stats ---
                    nc.vector.tensor_copy(g_sb[h][:], g_tile[:])
                    t2p = t2_ps.tile([128, 512], f32, name="t2p", tag="t2p")
                    for kc in range(2):
                        for cc in range(2):
                            nc.tensor.matmul(
                                t2p[:, kc * 256:(kc + 1) * 256],
                                g_sb[h][:, cc * 256 + kc * 128:cc * 256 + (kc + 1) * 128],
                                wq[:, 2 * h + cc, :],
                                start=(kc == 0 and cc == 0),
                                stop=(kc == 1 and cc == 1))
                    nc.vector.tensor_copy(t2_sb[h][:], t2p[:])
                    s_t = s_ps.tile([128, 512], f32, name="s", tag="s")
                    for qc in range(2):
                        for kc in range(2):
                            nc.tensor.matmul(
                                s_t[:, qc * 256:(qc + 1) * 256],
                                t2_sb[h][:, kc * 256 + qc * 128:kc * 256 + (qc + 1) * 128],
                                wk[:, kc, :],
                                start=(qc == 0 and kc == 0),
                                stop=(qc == 1 and kc == 1))
                    # stats from an SBUF copy (DVE allows only one PSUM input)
                    s16 = s16_pool.tile([128, 512], f16, name="s16", tag="s16")
                    nc.vector.tensor_copy(s16[:], s_t[:])
                    stat_h = stat_pool.tile([128, 4], f32, name="st", tag="st")
                    for cc in range(2):
                        s_ap = s16[:, cc * 256:(cc + 1) * 256]
                        nc.vector.reduce_sum(stat_h[:, cc:cc + 1], s_ap, axis=X)
                        nc.vector.tensor_mul(
                            scratch[:, cc * 256:(cc + 1) * 256], s_ap, s_ap)
                        nc.vector.reduce_sum(
                            stat_h[:, 2 + cc:3 + cc],
                            scratch[:, cc * 256:(cc + 1) * 256], axis=X)
                    yield
                    # --- P2: partition-sum, inorm scalars, broadcast ---
                    tbh = tb_ps.tile([128, 8], f32, name="tbh", tag="tbh")
                    nc.tensor.matmul(tbh[:1, 0:4], ones_col[:], stat_h[:],
                                     start=True, stop=True)
                    sch = stat_pool.tile([1, 16], f32, name="sc", tag="sc")
                    nc.vector.tensor_copy(sch[:1, 0:4], tbh[:1, 0:4])
                    nc.vector.tensor_add(sch[:1, 4:6], sch[:1, 0:4:2], sch[:1, 1:4:2])
                    nc.vector.tensor_scalar_mul(sch[:1, 6:8], sch[:1, 4:6], inv)
                    nc.vector.tensor_mul(sch[:1, 8:9], sch[:1, 6:7], sch[:1, 6:7])
                    nc.vector.tensor_sub(sch[:1, 9:10], sch[:1, 7:8], sch[:1, 8:9])
                    nc.vector.tensor_scalar_add(sch[:1, 10:11], sch[:1, 9:10], EPS_ADJ)
                    nc.scalar.sqrt(sch[:1, 11:12], sch[:1, 10:11])
                    nc.vector.reciprocal(sch[:1, 12:13], sch[:1, 11:12])
                    nc.vector.tensor_mul(sch[:1, 13:14], sch[:1, 6:7], sch[:1, 12:13])
                    nc.vector.tensor_scalar_mul(sch[:1, 14:15], sch[:1, 13:14], -1.0)
                    nc.tensor.matmul(tbh[:, 4:6], ones_row[:], sch[:1, 12:15:2],
                                     start=True, stop=True)
                    bch = stat_pool.tile([128, 2], f32, name="bc", tag="bc")
                    nc.vector.tensor_copy(bch[:], tbh[:, 4:6])
                    yield
                    # --- P3: exp with accumulated denominators ---
                    p = ps[h]
                    den = dens[h]
                    for cc in range(2):
                        nc.scalar.activation(
                            p[:, cc * 256:(cc + 1) * 256],
                            s_t[:, cc * 256:(cc + 1) * 256],
                            Exp, bias=bch[:, 1:2], scale=bch[:, 0:1],
                            accum_out=den[:, cc:cc + 1])
                        nc.vector.reciprocal(den[:, 2 + cc:3 + cc], den[:, cc:cc + 1])
                    yield
                    # --- P4: p^T and pv = (p Wvd)/den ---
                    for kc in range(2):
                        ptp = pt_ps.tile([128, 256], f16, name="ptp", tag="ptp")
                        for cc in range(2):
                            nc.tensor.transpose(
                                ptp[:, cc * 128:(cc + 1) * 128],
                                p[:, cc * 256 + kc * 128:cc * 256 + (kc + 1) * 128],
                                ident[:])
                        nc.vector.tensor_copy(pts[h * 2 + kc][:], ptp[:])
                    pvp = pv_ps.tile([128, 512], f32, name="pvp", tag="pvp")
                    for cc in range(2):
                        for kc in range(2):
                            nc.tensor.matmul(
                                pvp[:, cc * 256:(cc + 1) * 256],
                                pts[h * 2 + kc][:, cc * 128:(cc + 1) * 128],
                                wv[:, kc, :],
                                start=(cc == 0 and kc == 0),
                                stop=(cc == 1 and kc == 1))
                    for cc in range(2):
                        nc.vector.tensor_scalar_mul(
                            pv_sb[h][:, cc * 256:(cc + 1) * 256],
                            pvp[:, cc * 256:(cc + 1) * 256],
                            dens[h][:, 2 + cc:3 + cc])
                    yield

                def m_head(h, oh, evac_scalar):
                    """M[(h,kk),o-half] = sum_c pv[c,kk] WoutT[(h,c),o-half]."""
                    for kc2 in range(2):
                        mp = m_ps.tile([128, 512], f32, name="mp", tag="mp")
                        for cc in range(2):
                            nc.tensor.matmul(
                                mp[:],
                                pv_sb[h][:, cc * 256 + kc2 * 128:cc * 256 + (kc2 + 1) * 128],
                                wout[:, 2 * h + cc, oh * 512:(oh + 1) * 512],
                                start=(cc == 0), stop=(cc == 1))
                        dst = m_sb[:, 2 * h + kc2, oh * 512:(oh + 1) * 512]
                        if evac_scalar and not O_FP8:
                            nc.scalar.copy(dst, mp[:])
                        else:
                            nc.vector.tensor_copy(dst, mp[:])

                # ================= phase A: head-major G + chains =========
                gen_prev = None
                nsl = np_ // 4                     # 4 slabs of 4 slice-pairs
                for h in range(H):
                    g_tile = g_ps.tile([128, 512], f32, name="gp", tag="gp")
                    for sb in range(nsl):
                        # 512KB slabs (4KB contiguous per partition) keep the
                        # DMA queue efficient; small transfers bottleneck on
                        # per-DMA issue overhead at ~190GB/s
                        e1b = e1_pool.tile([128, 8, 256], emb_dt, name="e1", tag="e1")
                        nc.sync.dma_start(
                            e1b[:], e1tm_d[:, (h * np_ + sb * 4) * 2:(h * np_ + (sb + 1) * 4) * 2, :])
                        edb = ed_pool.tile([128, 8, 256], emb_dt, name="ed", tag="ed")
                        nc.sync.dma_start(
                            edb[:], edtm_d[:, (h * np_ + sb * 4) * 2:(h * np_ + (sb + 1) * 4) * 2, :])
                        if G_FP8:
                            for pr in range(4):
                                for cc in range(2):
                                    nc.tensor.matmul(
                                        g_tile[:, cc * 256:(cc + 1) * 256],
                                        e1b[:, 2 * pr:2 * pr + 2, cc * 128:(cc + 1) * 128],
                                        edb[:, 2 * pr:2 * pr + 2, :],
                                        start=(sb == 0 and pr == 0 and cc == 0),
                                        stop=(sb == nsl - 1 and pr == 3 and cc == 1),
                                        perf_mode=DR)
                        else:
                            for pr in range(4):
                                for half in range(2):
                                    for cc in range(2):
                                        nc.tensor.matmul(
                                            g_tile[:, cc * 256:(cc + 1) * 256],
                                            e1b[:, 2 * pr + half, cc * 128:(cc + 1) * 128],
                                            edb[:, 2 * pr + half, :],
                                            start=(sb == 0 and pr == 0 and half == 0 and cc == 0),
                                            stop=(sb == nsl - 1 and pr == 3 and half == 1 and cc == 1))
                        if gen_prev is not None and sb in (1, 2, 3):
                            next(gen_prev)
                        if h == 0 and sb == 1:
                            nc.sync.dma_start(wq[:], wq_d[:])
                            nc.sync.dma_start(wk[:], wk_d[:])
                            nc.sync.dma_start(wv[:], wv_d[:])
                    gen = chain(h, g_tile)
                    next(gen)                      # P1
                    gen_prev = gen

                # deferred phase-B loads, queued behind the G stream; all of
                # them hide inside the last head's ~10us chain window, in
                # order of first use (wout gates M, ekv0 gates O group 0)
                nc.sync.dma_start(wout[:, :, 0:512], wout_d[:, :, 0:512])
                nc.sync.dma_start(wout[:, :, 512:1024], wout_d[:, :, 512:1024])
                ekv_b0 = ekvB_pool.tile([128, 8, 512], cm_dt, name="ekv_b", tag="ekv_b")
                nc.sync.dma_start(ekv_b0[:], edcm_d[:, :, 0:512])
                ekv_b1 = None
                if ng > 1:
                    ekv_b1 = ekvB_pool.tile([128, 8, 512], cm_dt, name="ekv_b", tag="ekv_b")
                    nc.sync.dma_start(ekv_b1[:], edcm_d[:, :, 512:1024])

                # finish h3's chain, overlapped with M for the ready heads
                next(gen_prev)                     # P2
                for h in range(H - 1):
                    m_head(h, 0, evac_scalar=False)
                next(gen_prev)                     # P3 (exp on scalar)
                for h in range(H - 1):
                    m_head(h, 1, evac_scalar=False)
                next(gen_prev)                     # P4
                m_head(H - 1, 0, evac_scalar=False)
                m_head(H - 1, 1, evac_scalar=True)

            # ================= phase B: O = Ed_cm^T M =====================
            with ExitStack() as bctx:
                bpool = lambda **kw: bctx.enter_context(tc.tile_pool(**kw))
                o_ps = bpool(name="o_ps", bufs=4, space="PSUM")
                o_sb = bpool(name="o_sb", bufs=6)
                ekvB2_pool = bpool(name="ekvB2", bufs=3)
                for g in range(ng):
                    if g == 0:
                        ekv = ekv_b0
                    elif g == 1 and ekv_b1 is not None:
                        ekv = ekv_b1
                    else:
                        ekv = ekvB2_pool.tile([128, 8, 512], cm_dt, name="ekv2", tag="ekv2")
                        nc.sync.dma_start(ekv[:], edcm_d[:, :, g * 512:(g + 1) * 512])
                    for nsl in range(4):
                        ot = o_sb.tile([128, 1024], f16, name="ot", tag="ot")
                        for oh in range(2):
                            op = o_ps.tile([128, 512], f32, name="op", tag="op")
                            if O_FP8:
                                for jj in range(4):
                                    nc.tensor.matmul(
                                        op[:], ekv[:, 2 * jj:2 * jj + 2, nsl * 128:(nsl + 1) * 128],
                                        m_sb[:, 2 * jj:2 * jj + 2, oh * 512:(oh + 1) * 512],
                                        start=(jj == 0), stop=(jj == 3),
                                        perf_mode=DR)
                                # undo the 2^9 Wout prescale (exact)
                                if oh == 0:
                                    nc.vector.tensor_scalar_mul(ot[:, 0:512], op[:], 1.0 / 512.0)
                                else:
                                    nc.scalar.mul(ot[:, 512:1024], op[:], 1.0 / 512.0)
                            else:
                                for j in range(8):
                                    nc.tensor.matmul(
                                        op[:], ekv[:, j, nsl * 128:(nsl + 1) * 128],
                                        m_sb[:, j, oh * 512:(oh + 1) * 512],
                                        start=(j == 0), stop=(j == 7))
                                if oh == 0:
                                    nc.vector.tensor_copy(ot[:, 0:512], op[:])
                                else:
                                    nc.scalar.copy(ot[:, 512:1024], op[:])
                        r0 = g * 512 + nsl * 128
                        nc.sync.dma_start(out_d[r0:r0 + 128, :], ot[:])

    nc.compile()
    return nc


# ---------------- host-side data prep ----------------

def _emb_dt():
    if G_FP8:
        import ml_dtypes
        return ml_dtypes.float8_e4m3
    return np.float16


def _cm_dt():
    if O_FP8:
        import ml_dtypes
        return ml_dtypes.float8_e4m3
    return np.float16


def _prep_emb_hm(e):
    # [nt, 1024] -> [128, H*(nt//256)*2, 256]: head-major token slices;
    # A[p, (h, sp, sl01), cx] = e[(2*sp+sl01)*128+p, h*256+cx]
    nt = e.shape[0]
    a = e.reshape(nt // 256, 2, 128, 4, 256)          # [sp, sl01, p, h, cx]
    a = a.transpose(2, 3, 0, 1, 4).reshape(128, 8 * (nt // 256), 256)
    return np.ascontiguousarray(a.astype(_emb_dt()))


def _prep_embT(e):
    # [nt, 1024] -> [128, 8, nt]: partition p, chunk cc -> channel cc*128+p
    return np.ascontiguousarray(
        e.T.reshape(8, 128, -1).transpose(1, 0, 2).astype(_cm_dt()))


def _prep_wq(Wq):
    # [H, o, c] -> WqT [h, c, o] -> [128, (h,cc), 256]
    WqT = Wq.transpose(0, 2, 1)
    return np.ascontiguousarray(
        WqT.reshape(4, 2, 128, 256).transpose(2, 0, 1, 3).reshape(128, 8, 256)
        .astype(np.float16))


def _prep_wk(Wk):
    # [k, c] -> T [c, k] -> [128, cc, 256]  (WkdT chunks: rhs[p=k', j=ok])
    return np.ascontiguousarray(
        Wk.T.reshape(2, 128, 256).transpose(1, 0, 2).astype(np.float16))


def _prep_wv_native(Wv):
    # [kout, kin] native rows chunked: [128, kc, 256]  (rhs[p=k, j=kk])
    return np.ascontiguousarray(
        Wv.reshape(2, 128, 256).transpose(1, 0, 2).astype(np.float16))


def _prep_wout(Wo):
    # [o, C] with C=c*4+h -> Wo.T [C,o] -> head-major perm [h*256+c, o] -> chunks
    # O_FP8: prescale by 2^9 so M values sit in e4m3 normal range
    WoT = Wo.T.reshape(256, 4, 1024).transpose(1, 0, 2).reshape(1024, 1024)
    if O_FP8:
        WoT = WoT * 512.0
    return np.ascontiguousarray(
        WoT.reshape(8, 128, 1024).transpose(1, 0, 2).astype(np.float16))


def make_in_maps(inputs):
    f = lambda x: np.asarray(x, dtype=np.float32)
    emb1, emb_all = f(inputs["emb1"]), f(inputs["emb_all"])
    embd1, emb_alld = f(inputs["embd1"]), f(inputs["emb_alld"])
    branch_w = [
        (_prep_wq(f(inputs["Wq"])), _prep_wk(f(inputs["Wkd0"])),
         _prep_wv_native(f(inputs["Wvd0"])), _prep_wout(f(inputs["Wout"]))),
        (_prep_wq(f(inputs["Wqd"])), _prep_wk(f(inputs["Wk0"])),
         _prep_wv_native(f(inputs["Wv0"])), _prep_wout(f(inputs["Woutd"]))),
    ]
    in_maps = []
    for core in range(8):
        b, br = core % 4, core // 4
        if br == 0:
            eq, ekv = emb1[b], emb_alld[b]
        else:
            eq, ekv = embd1[b], emb_all[b]
        wq, wk, wv, wo = branch_w[br]
        in_maps.append({
            "e1tm": _prep_emb_hm(eq),
            "edtm": _prep_emb_hm(ekv),
            "edcm": _prep_embT(ekv),
            "wq": wq, "wk": wk, "wv": wv, "wout": wo,
        })
    return in_maps


_NC_CACHE = {}


def get_nc(n_tokens=N):
    if n_tokens not in _NC_CACHE:
        _NC_CACHE[n_tokens] = build_nc(n_tokens)
    return _NC_CACHE[n_tokens]


def run_on_hw(in_maps, trace=False):
    _ensure_axon_hooks()
    from concourse.bass_utils import run_bass_kernel_spmd
    nc = get_nc()
    return run_bass_kernel_spmd(nc, in_maps, list(range(len(in_maps))), trace=trace)


def kernel(**inputs):
    res = run_on_hw(make_in_maps(inputs), trace=False)
    O1 = np.stack([np.float32(res.results[b]["out"]) for b in range(4)])
    Od1 = np.stack([np.float32(res.results[4 + b]["out"]) for b in range(4)])
    return O1, Od1
